# revision 4
# baseline (speedup 1.0000x reference)
"""Multi-head attention Trainium2 kernel.

Problem: B=4, S=2048, D_MODEL=1024, H=16 heads, d_k=d_v=64.

Sharding (8 cores, no collectives): core c handles batch b=c//2 and head
group g=c%2 (8 heads). Each core computes its 8 heads' attention and the
partial output projection ctx @ Wo[g's rows]; the host sums the two
head-group partials per batch and adds the (folded) biases.

Math notes:
 - bk drops out of softmax exactly (adds a per-(q,head) constant to every
   score in a row).
 - bv and bo fold into a single host-side row vector: bo_eff = bo + bv@Wo
   (softmax weights sum to 1).
 - softmax is computed without max subtraction: weights are ~N(0, 0.02^2),
   so scores have std ~0.4 and |score| < ~3; exp is safe in fp32.
 - matmuls run in bf16 with fp32 PSUM accumulation.

Device layout (per core):
 - scores^T[s, q] per head: s on partitions (16 tiles of 128), q on free.
   lhsT = khT (dk x s-chunk), rhs = qhT (dk x q) -> K=64 matmuls; head
   pairs sit at partition halves 0-63 / 64-127 so pairs run concurrently
   on the row-tiled PE array.
 - exp on ScalarE reads scores PSUM (128x2048) and writes bf16 tiles.
 - ctx^T via lhsT = [vh | ones] (128 s-chunk x 65): row 64 of the PSUM
   output is the softmax denominator Z[q], obtained for free.
 - 1/Z broadcast across 64 partitions with a K=1 ones matmul; VectorE
   multiplies ctx rows and writes bf16 ctxT.
 - out[q, :] = sum_hc ctxT[:, hc, q-tile].T @ Wo chunk, fp32 out to HBM.
"""

import numpy as np
import ml_dtypes

import concourse.bass as bass
import concourse.bacc as bacc
import concourse.mybir as mybir
import concourse.tile as tile
from concourse.bass import ts

BF16 = mybir.dt.bfloat16
F32 = mybir.dt.float32

D_MODEL, D_K, D_V, N_HEADS = 1024, 64, 64, 16
B, S = 4, 2048
N_CORES = 8
NH = 8            # heads per core
HD = NH * D_V     # 512, stacked head dim per core
T = S             # tokens per core (one batch)
DC = 8            # D_MODEL / 128 chunks
TCN = 4           # token chunks of 512 for projections
SCN = 16          # s tiles of 128
QCN = 4           # q chunks of 512
HCN = 4           # hd chunks of 128 (2 heads each)


def build_nc(reps: int = 1):
    """Build the per-core Bass module. reps>1 wraps the body in a HW loop
    (used only for timing)."""
    nc = bacc.Bacc("TRN2", target_bir_lowering=False, debug=False)

    xq_d = nc.dram_tensor("xq_t", [128, DC, T], F32, kind="ExternalInput")
    xk_d = nc.dram_tensor("xk_t", [128, DC, T], F32, kind="ExternalInput")
    xv_d = nc.dram_tensor("xv_t", [128, DC, T], F32, kind="ExternalInput")
    wq_d = nc.dram_tensor("wq", [128, DC, HD], BF16, kind="ExternalInput")
    wk_d = nc.dram_tensor("wk", [128, DC, HD], BF16, kind="ExternalInput")
    wv_d = nc.dram_tensor("wv", [128, DC, HD], BF16, kind="ExternalInput")
    wo_d = nc.dram_tensor("wo", [128, HCN, D_MODEL], BF16, kind="ExternalInput")
    bq_d = nc.dram_tensor("bq", [128, HCN], F32, kind="ExternalInput")
    out_d = nc.dram_tensor("out", [SCN, 128, D_MODEL], F32, kind="ExternalOutput")

    with tile.TileContext(nc) as tc:
        def body():
            emit_body(nc, tc, xq_d, xk_d, xv_d, wq_d, wk_d, wv_d, wo_d, bq_d, out_d)

        if reps == 1:
            body()
        else:
            with tc.For_i(0, reps, 1):
                body()
    nc.compile()
    return nc


def emit_body(nc, tc, xq_d, xk_d, xv_d, wq_d, wk_d, wv_d, wo_d, bq_d, out_d):
    import contextlib

    ctx = contextlib.ExitStack()
    with ctx:
        # ---------------- persistent SBUF pools ----------------
        wpool = ctx.enter_context(tc.tile_pool(name="wpool", bufs=1))
        qkpool = ctx.enter_context(tc.tile_pool(name="qkpool", bufs=1))
        vpool = ctx.enter_context(tc.tile_pool(name="vpool", bufs=1))
        cpool = ctx.enter_context(tc.tile_pool(name="cpool", bufs=1))
        opool = ctx.enter_context(tc.tile_pool(name="opool", bufs=4))
        zpool = ctx.enter_context(tc.tile_pool(name="zpool", bufs=4))

        wq_sb = wpool.tile([128, DC, HD], BF16, tag="wq")
        wk_sb = wpool.tile([128, DC, HD], BF16, tag="wk")
        wv_sb = wpool.tile([128, DC, HD], BF16, tag="wv")
        wo_sb = wpool.tile([128, HCN, D_MODEL], BF16, tag="wo")
        bq_sb = wpool.tile([128, HCN], F32, tag="bq")
        ones64 = wpool.tile([1, 64], BF16, tag="ones64")

        nc.sync.dma_start(wq_sb[:], wq_d[:])
        nc.sync.dma_start(wk_sb[:], wk_d[:])
        nc.sync.dma_start(wv_sb[:], wv_d[:])
        nc.sync.dma_start(wo_sb[:], wo_d[:])
        nc.sync.dma_start(bq_sb[:], bq_d[:])
        nc.vector.memset(ones64[:], 1.0)

        qhT = qkpool.tile([128, HCN, T], BF16, tag="qhT")  # [hd%128, hd//128, t]
        khT = qkpool.tile([128, HCN, T], BF16, tag="khT")
        vha = vpool.tile([128, SCN, NH, D_V + 1], BF16, tag="vha")  # [s%128, s//128, h, dv|1]
        ctxT = cpool.tile([128, HCN, T], BF16, tag="ctxT")

        nc.vector.memset(vha[:, :, :, D_V : D_V + 1], 1.0)

        # ---------------- phase A: projections ----------------
        with (
            tc.tile_pool(name="xf32", bufs=2) as xf32p,
            tc.tile_pool(name="xbf", bufs=2) as xbfp,
            tc.tile_pool(name="pp", bufs=6, space="PSUM") as pp,
        ):
            # qhT / khT: psum (128 hd-chunk, 512 t) = W[dc,hc].T @ xT[dc, tc]
            for x_d, w_sb, dst, has_bias in (
                (xq_d, wq_sb, qhT, True),
                (xk_d, wk_sb, khT, False),
            ):
                for tc_i in range(TCN):
                    xf = xf32p.tile([128, DC, 512], F32, tag="xf", name="xf_t")
                    nc.sync.dma_start(xf[:], x_d[:, :, ts(tc_i, 512)])
                    xb = xbfp.tile([128, DC, 512], BF16, tag="xb", name="xb_t")
                    nc.vector.tensor_copy(xb[:], xf[:])
                    ptiles = [pp.tile([128, 512], F32, tag="pp", name="pp_t") for _ in range(HCN)]
                    for dc in range(DC):
                        for hc in range(HCN):
                            nc.tensor.matmul(
                                ptiles[hc][:],
                                lhsT=w_sb[:, dc, ts(hc, 128)],
                                rhs=xb[:, dc, :],
                                start=(dc == 0),
                                stop=(dc == DC - 1),
                            )
                    for hc in range(HCN):
                        if has_bias:
                            nc.vector.tensor_scalar_add(
                                dst[:, hc, ts(tc_i, 512)], ptiles[hc][:], bq_sb[:, hc : hc + 1]
                            )
                        else:
                            nc.scalar.copy(dst[:, hc, ts(tc_i, 512)], ptiles[hc][:])

            # vh (normal layout): psum (128 s-tile, 512 hd) = xT[dc, s-tile].T @ W[dc]
            for tc_i in range(TCN):
                xf = xf32p.tile([128, DC, 512], F32, tag="xf", name="xf_t")
                nc.sync.dma_start(xf[:], xv_d[:, :, ts(tc_i, 512)])
                xb = xbfp.tile([128, DC, 512], BF16, tag="xb", name="xb_t")
                nc.vector.tensor_copy(xb[:], xf[:])
                for sj in range(4):
                    sc = tc_i * 4 + sj
                    pv = pp.tile([128, 512], F32, tag="pp", name="pv_t")
                    for dc in range(DC):
                        nc.tensor.matmul(
                            pv[:],
                            lhsT=xb[:, dc, ts(sj, 128)],
                            rhs=wv_sb[:, dc, :],
                            start=(dc == 0),
                            stop=(dc == DC - 1),
                        )
                    nc.vector.tensor_copy(
                        vha[:, sc, :, 0:D_V],
                        pv[:].rearrange("p (h d) -> p h d", d=D_V),
                    )

        # ---------------- phase B: attention ----------------
        with (
            tc.tile_pool(name="expool", bufs=18) as expool,
            tc.tile_pool(name="ps", bufs=1, space="PSUM") as ps,
            tc.tile_pool(name="cp", bufs=2, space="PSUM") as cp,
            tc.tile_pool(name="zb", bufs=2, space="PSUM") as zb,
        ):
            exp_tiles = {}

            def scores_exp(h):
                hc, pb = h // 2, (h % 2) * 64
                for sc in range(SCN):
                    s_ps = ps.tile([128, T], F32, tag="ps", name="s_ps")
                    for qc in range(QCN):
                        nc.tensor.matmul(
                            s_ps[:, ts(qc, 512)],
                            lhsT=khT[pb : pb + 64, hc, ts(sc, 128)],
                            rhs=qhT[pb : pb + 64, hc, ts(qc, 512)],
                            start=True,
                            stop=True,
                        )
                    e = expool.tile([128, T], BF16, tag="exp", name="exp_t")
                    nc.scalar.activation(
                        e[:], s_ps[:], mybir.ActivationFunctionType.Exp, scale=0.125
                    )
                    exp_tiles[(h, sc)] = e

            def ctx_head(h):
                hc, pb = h // 2, (h % 2) * 64
                for qc in range(QCN):
                    c_ps = cp.tile([128, 512], F32, tag="cp", name="c_ps")
                    for sc in range(SCN):
                        nc.tensor.matmul(
                            c_ps[0 : D_V + 1, :],
                            lhsT=vha[:, sc, h, :],
                            rhs=exp_tiles[(h, sc)][:, ts(qc, 512)],
                            start=(sc == 0),
                            stop=(sc == SCN - 1),
                        )
                    rz = zpool.tile([1, 512], F32, tag="rz", name="rz_t")
                    nc.vector.reciprocal(rz[:], c_ps[D_V : D_V + 1, :])
                    rzb = zpool.tile([1, 512], BF16, tag="rzb", name="rzb_t")
                    nc.vector.tensor_copy(rzb[:], rz[:])
                    bc = zb.tile([64, 512], F32, tag="zb", name="bc_t")
                    nc.tensor.matmul(bc[:], lhsT=ones64[:], rhs=rzb[:], start=True, stop=True)
                    # TT can read only one PSUM operand -> stage broadcast in SBUF
                    bc_sb = zpool.tile([64, 512], F32, tag="bcs", name="bc_sb")
                    nc.vector.tensor_copy(bc_sb[:], bc[:])
                    nc.vector.tensor_mul(
                        ctxT[pb : pb + 64, hc, ts(qc, 512)], c_ps[0:D_V, :], bc_sb[:]
                    )
                # free this head's exp tiles from our dict (slots recycled by pool)
                for sc in range(SCN):
                    del exp_tiles[(h, sc)]

            for h in range(NH):
                scores_exp(h)
                if h > 0:
                    ctx_head(h - 1)
            ctx_head(NH - 1)

            # ---------------- output projection ----------------
            for qt in range(SCN):
                for d2 in range(2):
                    po = cp.tile([128, 512], F32, tag="cp", name="po_t")
                    for hc in range(HCN):
                        nc.tensor.matmul(
                            po[:],
                            lhsT=ctxT[:, hc, ts(qt, 128)],
                            rhs=wo_sb[:, hc, ts(d2, 512)],
                            start=(hc == 0),
                            stop=(hc == HCN - 1),
                        )
                    o_sb = opool.tile([128, 512], F32, tag="o", name="o_sb")
                    nc.vector.tensor_copy(o_sb[:], po[:])
                    nc.sync.dma_start(out_d[qt, :, ts(d2, 512)], o_sb[:])


# ---------------------------------------------------------------------------
# host side
# ---------------------------------------------------------------------------

_NC_CACHE = {}


def _get_nc(reps: int = 1):
    if reps not in _NC_CACHE:
        _NC_CACHE[reps] = build_nc(reps)
    return _NC_CACHE[reps]


def _to_bf16(a):
    return np.ascontiguousarray(a).astype(ml_dtypes.bfloat16)


def make_in_maps(q, k, v, Wq, bq, Wk, bk, Wv, bv, Wo, bo):
    """Build the per-core input maps (host-side sharding + layout)."""
    in_maps = []
    for c in range(N_CORES):
        b = c // 2
        hg = c % 2
        hs = slice(hg * NH, hg * NH + NH)

        def xt(x):
            # (S, D) -> [p, dc, t] with D = dc*128 + p
            return np.ascontiguousarray(
                np.asarray(x, np.float32).T.reshape(DC, 128, T).transpose(1, 0, 2)
            )

        def wproj(W):
            # (8, 1024, 64) -> [p, dc, hd]  (hd = h*64+dv, D = dc*128+p)
            Wc = np.asarray(W[hs], np.float32).transpose(1, 0, 2).reshape(D_MODEL, HD)
            return _to_bf16(Wc.reshape(DC, 128, HD).transpose(1, 0, 2))

        wo_c = np.asarray(Wo[hg * HD : (hg + 1) * HD], np.float32)  # (512, 1024)
        bq_c = np.asarray(bq[hs], np.float32).reshape(HD)  # (512,)

        in_maps.append(
            {
                "xq_t": xt(q[b]),
                "xk_t": xt(k[b]),
                "xv_t": xt(v[b]),
                "wq": wproj(Wq),
                "wk": wproj(Wk),
                "wv": wproj(Wv),
                "wo": _to_bf16(wo_c.reshape(HCN, 128, D_MODEL).transpose(1, 0, 2)),
                "bq": np.ascontiguousarray(bq_c.reshape(HCN, 128).T),
            }
        )
    return in_maps


def combine_outputs(results, bv, Wo, bo):
    """results: list of 8 dicts with 'out' (16,128,1024). Returns (B,S,D)."""
    bo_eff = np.asarray(bo, np.float32) + np.asarray(bv, np.float32).reshape(-1) @ np.asarray(
        Wo, np.float32
    )
    out = np.empty((B, S, D_MODEL), np.float32)
    for b in range(B):
        p0 = results[2 * b]["out"].reshape(S, D_MODEL)
        p1 = results[2 * b + 1]["out"].reshape(S, D_MODEL)
        out[b] = p0 + p1 + bo_eff
    return out


def kernel(q, k, v, Wq, bq, Wk, bk, Wv, bv, Wo, bo):
    from concourse.bass_utils import run_bass_kernel_spmd

    nc = _get_nc(1)
    in_maps = make_in_maps(q, k, v, Wq, bq, Wk, bk, Wv, bv, Wo, bo)
    res = run_bass_kernel_spmd(nc, in_maps, core_ids=list(range(N_CORES)))
    return combine_outputs(res.results, bv, Wo, bo)


# revision 6
# speedup vs baseline: 1.4461x; 1.4461x over previous
"""Multi-head attention Trainium2 kernel.

Problem: B=4, S=2048, D_MODEL=1024, H=16 heads, d_k=d_v=64.

Sharding (8 cores, no collectives): core c handles batch b=c//2 and head
group g=c%2 (8 heads). Each core computes its 8 heads' attention and the
partial output projection ctx @ Wo[g's rows]; the host sums the two
head-group partials per batch and adds the (folded) biases.

Math notes:
 - bk drops out of softmax exactly (adds a per-(q,head) constant to every
   score in a row).
 - bv and bo fold into a single host-side row vector: bo_eff = bo + bv@Wo
   (softmax weights sum to 1).
 - softmax is computed without max subtraction: weights are ~N(0, 0.02^2),
   so scores have std ~0.4 and |score| < ~3; exp is safe in fp32.
 - matmuls run in bf16 with fp32 PSUM accumulation.

Device layout (per core):
 - scores^T[s, q] per head: s on partitions (16 tiles of 128), q on free.
   lhsT = khT (dk x s-chunk), rhs = qhT (dk x q) -> K=64 matmuls; head
   pairs sit at partition halves 0-63 / 64-127 so pairs run concurrently
   on the row-tiled PE array.
 - exp on ScalarE reads scores PSUM (128x2048) and writes bf16 tiles.
 - ctx^T via lhsT = [vh | ones] (128 s-chunk x 65): row 64 of the PSUM
   output is the softmax denominator Z[q], obtained for free.
 - 1/Z broadcast across 64 partitions with a K=1 ones matmul; VectorE
   multiplies ctx rows and writes bf16 ctxT.
 - out[q, :] = sum_hc ctxT[:, hc, q-tile].T @ Wo chunk, fp32 out to HBM.
"""

import numpy as np
import ml_dtypes

import concourse.bass as bass
import concourse.bacc as bacc
import concourse.mybir as mybir
import concourse.tile as tile
from concourse.bass import ts

BF16 = mybir.dt.bfloat16
F32 = mybir.dt.float32

D_MODEL, D_K, D_V, N_HEADS = 1024, 64, 64, 16
B, S = 4, 2048
N_CORES = 8
NH = 8            # heads per core
HD = NH * D_V     # 512, stacked head dim per core
T = S             # tokens per core (one batch)
DC = 8            # D_MODEL / 128 chunks
TCN = 4           # token chunks of 512 for projections
SCN = 16          # s tiles of 128
QCN = 4           # q chunks of 512
HCN = 4           # hd chunks of 128 (2 heads each)


def build_nc(reps: int = 1):
    """Build the per-core Bass module. reps>1 wraps the body in a HW loop
    (used only for timing)."""
    nc = bacc.Bacc("TRN2", target_bir_lowering=False, debug=False)

    xq_d = nc.dram_tensor("xq_t", [128, DC, T], F32, kind="ExternalInput")
    xk_d = nc.dram_tensor("xk_t", [128, DC, T], F32, kind="ExternalInput")
    xv_d = nc.dram_tensor("xv_t", [128, DC, T], F32, kind="ExternalInput")
    wq_d = nc.dram_tensor("wq", [128, DC, HD], BF16, kind="ExternalInput")
    wk_d = nc.dram_tensor("wk", [128, DC, HD], BF16, kind="ExternalInput")
    wv_d = nc.dram_tensor("wv", [128, DC, HD], BF16, kind="ExternalInput")
    wo_d = nc.dram_tensor("wo", [128, HCN, D_MODEL], BF16, kind="ExternalInput")
    bq_d = nc.dram_tensor("bq", [128, HCN], F32, kind="ExternalInput")
    out_d = nc.dram_tensor("out", [SCN, 128, D_MODEL], F32, kind="ExternalOutput")

    with tile.TileContext(nc) as tc:
        def body():
            emit_body(nc, tc, xq_d, xk_d, xv_d, wq_d, wk_d, wv_d, wo_d, bq_d, out_d)

        if reps == 1:
            body()
        else:
            with tc.For_i(0, reps, 1):
                body()
    nc.compile()
    return nc


def emit_body(nc, tc, xq_d, xk_d, xv_d, wq_d, wk_d, wv_d, wo_d, bq_d, out_d):
    import contextlib

    ctx = contextlib.ExitStack()
    with ctx:
        # ---------------- persistent SBUF pools ----------------
        wpool = ctx.enter_context(tc.tile_pool(name="wpool", bufs=1))
        qkpool = ctx.enter_context(tc.tile_pool(name="qkpool", bufs=1))
        vpool = ctx.enter_context(tc.tile_pool(name="vpool", bufs=1))
        cpool = ctx.enter_context(tc.tile_pool(name="cpool", bufs=1))
        opool = ctx.enter_context(tc.tile_pool(name="opool", bufs=4))
        zpool = ctx.enter_context(tc.tile_pool(name="zpool", bufs=4))

        wq_sb = wpool.tile([128, DC, HD], BF16, tag="wq")
        wk_sb = wpool.tile([128, DC, HD], BF16, tag="wk")
        wv_sb = wpool.tile([128, DC, HD], BF16, tag="wv")
        wo_sb = wpool.tile([128, HCN, D_MODEL], BF16, tag="wo")
        bq_sb = wpool.tile([128, HCN], F32, tag="bq")
        ones64 = wpool.tile([1, 64], BF16, tag="ones64")

        nc.sync.dma_start(wq_sb[:], wq_d[:])
        nc.sync.dma_start(wk_sb[:], wk_d[:])
        nc.sync.dma_start(wv_sb[:], wv_d[:])
        nc.sync.dma_start(wo_sb[:], wo_d[:])
        nc.sync.dma_start(bq_sb[:], bq_d[:])
        nc.vector.memset(ones64[:], 1.0)

        qhT = qkpool.tile([128, HCN, T], BF16, tag="qhT")  # [hd%128, hd//128, t]
        khT = qkpool.tile([128, HCN, T], BF16, tag="khT")
        vha = vpool.tile([128, SCN, NH, D_V + 1], BF16, tag="vha")  # [s%128, s//128, h, dv|1]
        ctxT = cpool.tile([128, HCN, T], BF16, tag="ctxT")

        nc.vector.memset(vha[:, :, :, D_V : D_V + 1], 1.0)

        # ---------------- phase A: q/k projections ----------------
        with (
            tc.tile_pool(name="xf32", bufs=2) as xf32p,
            tc.tile_pool(name="xbf", bufs=2) as xbfp,
            tc.tile_pool(name="pp", bufs=6, space="PSUM") as pp,
        ):
            # qhT / khT: psum (128 hd-chunk, 512 t) = W[dc,hc].T @ xT[dc, tc]
            for x_d, w_sb, dst, has_bias in (
                (xq_d, wq_sb, qhT, True),
                (xk_d, wk_sb, khT, False),
            ):
                for tc_i in range(TCN):
                    xf = xf32p.tile([128, DC, 512], F32, tag="xf", name="xf_t")
                    nc.sync.dma_start(xf[:], x_d[:, :, ts(tc_i, 512)])
                    xb = xbfp.tile([128, DC, 512], BF16, tag="xb", name="xb_t")
                    nc.vector.tensor_copy(xb[:], xf[:])
                    ptiles = [pp.tile([128, 512], F32, tag="pp", name="pp_t") for _ in range(HCN)]
                    for dc in range(DC):
                        for hc in range(HCN):
                            nc.tensor.matmul(
                                ptiles[hc][:],
                                lhsT=w_sb[:, dc, ts(hc, 128)],
                                rhs=xb[:, dc, :],
                                start=(dc == 0),
                                stop=(dc == DC - 1),
                            )
                    for hc in range(HCN):
                        if has_bias:
                            nc.vector.tensor_scalar_add(
                                dst[:, hc, ts(tc_i, 512)], ptiles[hc][:], bq_sb[:, hc : hc + 1]
                            )
                        else:
                            nc.scalar.copy(dst[:, hc, ts(tc_i, 512)], ptiles[hc][:])

        # ---------------- phase B: v projection + attention ----------------
        with (
            tc.tile_pool(name="vstage", bufs=2) as vstage,
            tc.tile_pool(name="vstageb", bufs=2) as vstageb,
            tc.tile_pool(name="expool", bufs=20) as expool,
            tc.tile_pool(name="ps", bufs=2, space="PSUM") as ps,
            tc.tile_pool(name="cp", bufs=2, space="PSUM") as cp,
            tc.tile_pool(name="zb", bufs=1, space="PSUM") as zb,
            tc.tile_pool(name="vp", bufs=1, space="PSUM") as vp,
        ):
            exp_tiles = {}

            def vh_proj():
                # vh (normal layout): psum (128 s-tile, 512 hd) = xT[dc, s].T @ W[dc]
                for sc in range(SCN):
                    xf = vstage.tile([128, DC, 128], F32, tag="vxf", name="vxf_t")
                    nc.sync.dma_start(xf[:], xv_d[:, :, ts(sc, 128)])
                    xb = vstageb.tile([128, DC, 128], BF16, tag="vxb", name="vxb_t")
                    nc.vector.tensor_copy(xb[:], xf[:])
                    pv = vp.tile([128, 512], F32, tag="vp", name="pv_t")
                    for dc in range(DC):
                        nc.tensor.matmul(
                            pv[:],
                            lhsT=xb[:, dc, :],
                            rhs=wv_sb[:, dc, :],
                            start=(dc == 0),
                            stop=(dc == DC - 1),
                        )
                    nc.vector.tensor_copy(
                        vha[:, sc, :, 0:D_V],
                        pv[:].rearrange("p (h d) -> p h d", d=D_V),
                    )

            def scores_exp_pair(p):
                # pair p = heads (2p, 2p+1) = hc p, partition halves 0/64.
                # One psum tile holds both heads for one 512-q chunk; the two
                # K=64 matmuls target row groups 0-63 / 64-127 and run
                # concurrently on the tiled PE array.
                for qc in range(QCN):
                    for sc in range(SCN):
                        s_ps = ps.tile([128, 1024], F32, tag="ps", name="s_ps")
                        for hl in range(2):
                            pb = hl * 64
                            nc.tensor.matmul(
                                s_ps[:, ts(hl, 512)],
                                lhsT=khT[pb : pb + 64, p, ts(sc, 128)],
                                rhs=qhT[pb : pb + 64, p, ts(qc, 512)],
                                start=True,
                                stop=True,
                            )
                        e = expool.tile([128, 1024], BF16, tag="exp", name="exp_t")
                        nc.scalar.activation(
                            e[:], s_ps[:], mybir.ActivationFunctionType.Exp, scale=0.125
                        )
                        exp_tiles[(p, sc, qc)] = e

            def ctx_pair(p):
                for qc in range(QCN):
                    for hl in range(2):
                        h = 2 * p + hl
                        pb = hl * 64
                        c_ps = cp.tile([128, 512], F32, tag="cp", name="c_ps")
                        for sc in range(SCN):
                            nc.tensor.matmul(
                                c_ps[0 : D_V + 1, :],
                                lhsT=vha[:, sc, h, :],
                                rhs=exp_tiles[(p, sc, qc)][:, ts(hl, 512)],
                                start=(sc == 0),
                                stop=(sc == SCN - 1),
                            )
                        rz = zpool.tile([1, 512], F32, tag="rz", name="rz_t")
                        nc.vector.reciprocal(rz[:], c_ps[D_V : D_V + 1, :])
                        rzb = zpool.tile([1, 512], BF16, tag="rzb", name="rzb_t")
                        nc.vector.tensor_copy(rzb[:], rz[:])
                        bc = zb.tile([64, 512], F32, tag="zb", name="bc_t")
                        nc.tensor.matmul(
                            bc[:], lhsT=ones64[:], rhs=rzb[:], start=True, stop=True
                        )
                        # TT can read only one PSUM operand -> stage bcast in SBUF
                        bc_sb = zpool.tile([64, 512], F32, tag="bcs", name="bc_sb")
                        nc.vector.tensor_copy(bc_sb[:], bc[:])
                        nc.vector.tensor_mul(
                            ctxT[pb : pb + 64, p, ts(qc, 512)], c_ps[0:D_V, :], bc_sb[:]
                        )
                    for sc in range(SCN):
                        del exp_tiles[(p, sc, qc)]

            vh_proj()
            for p in range(NH // 2):
                scores_exp_pair(p)
                if p > 0:
                    ctx_pair(p - 1)
            ctx_pair(NH // 2 - 1)

            # ---------------- output projection ----------------
            for qt in range(SCN):
                potiles = [
                    cp.tile([128, 512], F32, tag="cp", name="po_t") for _ in range(2)
                ]
                for hc in range(HCN):
                    for d2 in range(2):
                        nc.tensor.matmul(
                            potiles[d2][:],
                            lhsT=ctxT[:, hc, ts(qt, 128)],
                            rhs=wo_sb[:, hc, ts(d2, 512)],
                            start=(hc == 0),
                            stop=(hc == HCN - 1),
                        )
                for d2 in range(2):
                    o_sb = opool.tile([128, 512], F32, tag="o", name="o_sb")
                    nc.vector.tensor_copy(o_sb[:], potiles[d2][:])
                    nc.sync.dma_start(out_d[qt, :, ts(d2, 512)], o_sb[:])


# ---------------------------------------------------------------------------
# host side
# ---------------------------------------------------------------------------

_NC_CACHE = {}


def _get_nc(reps: int = 1):
    if reps not in _NC_CACHE:
        _NC_CACHE[reps] = build_nc(reps)
    return _NC_CACHE[reps]


def _to_bf16(a):
    return np.ascontiguousarray(a).astype(ml_dtypes.bfloat16)


def make_in_maps(q, k, v, Wq, bq, Wk, bk, Wv, bv, Wo, bo):
    """Build the per-core input maps (host-side sharding + layout)."""
    in_maps = []
    for c in range(N_CORES):
        b = c // 2
        hg = c % 2
        hs = slice(hg * NH, hg * NH + NH)

        def xt(x):
            # (S, D) -> [p, dc, t] with D = dc*128 + p
            return np.ascontiguousarray(
                np.asarray(x, np.float32).T.reshape(DC, 128, T).transpose(1, 0, 2)
            )

        def wproj(W):
            # (8, 1024, 64) -> [p, dc, hd]  (hd = h*64+dv, D = dc*128+p)
            Wc = np.asarray(W[hs], np.float32).transpose(1, 0, 2).reshape(D_MODEL, HD)
            return _to_bf16(Wc.reshape(DC, 128, HD).transpose(1, 0, 2))

        wo_c = np.asarray(Wo[hg * HD : (hg + 1) * HD], np.float32)  # (512, 1024)
        bq_c = np.asarray(bq[hs], np.float32).reshape(HD)  # (512,)

        in_maps.append(
            {
                "xq_t": xt(q[b]),
                "xk_t": xt(k[b]),
                "xv_t": xt(v[b]),
                "wq": wproj(Wq),
                "wk": wproj(Wk),
                "wv": wproj(Wv),
                "wo": _to_bf16(wo_c.reshape(HCN, 128, D_MODEL).transpose(1, 0, 2)),
                "bq": np.ascontiguousarray(bq_c.reshape(HCN, 128).T),
            }
        )
    return in_maps


def combine_outputs(results, bv, Wo, bo):
    """results: list of 8 dicts with 'out' (16,128,1024). Returns (B,S,D)."""
    bo_eff = np.asarray(bo, np.float32) + np.asarray(bv, np.float32).reshape(-1) @ np.asarray(
        Wo, np.float32
    )
    out = np.empty((B, S, D_MODEL), np.float32)
    for b in range(B):
        p0 = results[2 * b]["out"].reshape(S, D_MODEL)
        p1 = results[2 * b + 1]["out"].reshape(S, D_MODEL)
        out[b] = p0 + p1 + bo_eff
    return out


def kernel(q, k, v, Wq, bq, Wk, bk, Wv, bv, Wo, bo):
    from concourse.bass_utils import run_bass_kernel_spmd

    nc = _get_nc(1)
    in_maps = make_in_maps(q, k, v, Wq, bq, Wk, bk, Wv, bv, Wo, bo)
    res = run_bass_kernel_spmd(nc, in_maps, core_ids=list(range(N_CORES)))
    return combine_outputs(res.results, bv, Wo, bo)


# revision 11
# speedup vs baseline: 1.6001x; 1.1065x over previous
"""Multi-head attention Trainium2 kernel.

Problem: B=4, S=2048, D_MODEL=1024, H=16 heads, d_k=d_v=64.

Sharding (8 cores, no collectives): core c handles batch b=c//2 and head
group g=c%2 (8 heads). Each core computes its 8 heads' attention and the
partial output projection ctx @ Wo[g's rows]; the host sums the two
head-group partials per batch and adds the (folded) biases.

Math notes:
 - bk drops out of softmax exactly (adds a per-(q,head) constant to every
   score in a row).
 - bv and bo fold into a single host-side row vector: bo_eff = bo + bv@Wo
   (softmax weights sum to 1).
 - softmax is computed without max subtraction: weights are ~N(0, 0.02^2),
   so scores have std ~0.4 and |score| < ~3; exp is safe in fp32.
 - matmuls run in bf16 with fp32 PSUM accumulation.

Device layout (per core):
 - scores^T[s, q] per head: s on partitions (16 tiles of 128), q on free.
   lhsT = khT (dk x s-chunk), rhs = qhT (dk x q) -> K=64 matmuls; head
   pairs sit at partition halves 0-63 / 64-127 so pairs run concurrently
   on the row-tiled PE array.
 - exp on ScalarE reads scores PSUM (128x2048) and writes bf16 tiles.
 - ctx^T via lhsT = [vh | ones] (128 s-chunk x 65): row 64 of the PSUM
   output is the softmax denominator Z[q], obtained for free.
 - 1/Z broadcast across 64 partitions with a K=1 ones matmul; VectorE
   multiplies ctx rows and writes bf16 ctxT.
 - out[q, :] = sum_hc ctxT[:, hc, q-tile].T @ Wo chunk, fp32 out to HBM.
"""

import numpy as np
import ml_dtypes

import concourse.bass as bass
import concourse.bacc as bacc
import concourse.mybir as mybir
import concourse.tile as tile
from concourse.bass import ts

BF16 = mybir.dt.bfloat16
F32 = mybir.dt.float32

D_MODEL, D_K, D_V, N_HEADS = 1024, 64, 64, 16
B, S = 4, 2048
N_CORES = 8
NH = 8            # heads per core
HD = NH * D_V     # 512, stacked head dim per core
T = S             # tokens per core (one batch)
DC = 8            # D_MODEL / 128 chunks
TCN = 4           # token chunks of 512 for projections
SCN = 16          # s tiles of 128
QCN = 4           # q chunks of 512
HCN = 4           # hd chunks of 128 (2 heads each)


def build_nc(reps: int = 1, phases: str = "all"):
    """Build the per-core Bass module. reps>1 wraps the body in a HW loop
    (used only for timing). phases in {"all","proj","attn"} for perf probes."""
    nc = bacc.Bacc("TRN2", target_bir_lowering=False, debug=False)

    xq_d = nc.dram_tensor("xq_t", [128, DC, T], F32, kind="ExternalInput")
    xk_d = nc.dram_tensor("xk_t", [128, DC, T], F32, kind="ExternalInput")
    xv_d = nc.dram_tensor("xv_t", [128, DC, T], F32, kind="ExternalInput")
    wq_d = nc.dram_tensor("wq", [128, DC, HD], BF16, kind="ExternalInput")
    wk_d = nc.dram_tensor("wk", [128, DC, HD], BF16, kind="ExternalInput")
    wv_d = nc.dram_tensor("wv", [128, DC, HD], BF16, kind="ExternalInput")
    wo_d = nc.dram_tensor("wo", [128, HCN, D_MODEL], BF16, kind="ExternalInput")
    bq_d = nc.dram_tensor("bq", [128, HCN], F32, kind="ExternalInput")
    out_d = nc.dram_tensor("out", [SCN, 128, D_MODEL], F32, kind="ExternalOutput")

    with tile.TileContext(nc) as tc:
        def body():
            emit_body(nc, tc, xq_d, xk_d, xv_d, wq_d, wk_d, wv_d, wo_d, bq_d, out_d, phases)

        if reps == 1:
            body()
        else:
            with tc.For_i(0, reps, 1):
                body()
    nc.compile()
    return nc


def emit_body(nc, tc, xq_d, xk_d, xv_d, wq_d, wk_d, wv_d, wo_d, bq_d, out_d, phases="all"):
    import contextlib

    ctx = contextlib.ExitStack()
    with ctx:
        # ---------------- persistent SBUF pools ----------------
        wpool = ctx.enter_context(tc.tile_pool(name="wpool", bufs=1))
        qkpool = ctx.enter_context(tc.tile_pool(name="qkpool", bufs=1))
        vpool = ctx.enter_context(tc.tile_pool(name="vpool", bufs=1))
        cpool = ctx.enter_context(tc.tile_pool(name="cpool", bufs=1))
        opool = ctx.enter_context(tc.tile_pool(name="opool", bufs=4))
        zpool = ctx.enter_context(tc.tile_pool(name="zpool", bufs=4))

        wq_sb = wpool.tile([128, DC, HD], BF16, tag="wq")
        wk_sb = wpool.tile([128, DC, HD], BF16, tag="wk")
        wv_sb = wpool.tile([128, DC, HD], BF16, tag="wv")
        wo_sb = wpool.tile([128, HCN, D_MODEL], BF16, tag="wo")
        bq_sb = wpool.tile([128, HCN], F32, tag="bq")
        ones64 = wpool.tile([1, 64], BF16, tag="ones64")

        nc.sync.dma_start(wq_sb[:], wq_d[:])
        nc.sync.dma_start(wk_sb[:], wk_d[:])
        nc.sync.dma_start(wv_sb[:], wv_d[:])
        nc.sync.dma_start(wo_sb[:], wo_d[:])
        nc.sync.dma_start(bq_sb[:], bq_d[:])
        nc.vector.memset(ones64[:], 1.0)

        qhT = qkpool.tile([128, HCN, T], BF16, tag="qhT")  # [hd%128, hd//128, t]
        khT = qkpool.tile([128, HCN, T], BF16, tag="khT")
        vha = vpool.tile([128, SCN, NH, D_V + 1], BF16, tag="vha")  # [s%128, s//128, h, dv|1]
        ctxT = cpool.tile([128, HCN, T], BF16, tag="ctxT")

        nc.vector.memset(vha[:, :, :, D_V : D_V + 1], 1.0)

        # ---------------- phase A: q/k projections ----------------
        if phases in ("all", "proj"):
         with (
            tc.tile_pool(name="xf32", bufs=2) as xf32p,
            tc.tile_pool(name="xbf", bufs=2) as xbfp,
            tc.tile_pool(name="pp", bufs=6, space="PSUM") as pp,
        ):
            # qhT / khT: psum (128 hd-chunk, 512 t) = W[dc,hc].T @ xT[dc, tc]
            for x_d, w_sb, dst, has_bias in (
                (xq_d, wq_sb, qhT, True),
                (xk_d, wk_sb, khT, False),
            ):
                for tc_i in range(TCN):
                    xf = xf32p.tile([128, DC, 512], F32, tag="xf", name="xf_t")
                    nc.sync.dma_start(xf[:], x_d[:, :, ts(tc_i, 512)])
                    xb = xbfp.tile([128, DC, 512], BF16, tag="xb", name="xb_t")
                    nc.vector.tensor_copy(xb[:], xf[:])
                    ptiles = [pp.tile([128, 512], F32, tag="pp", name="pp_t") for _ in range(HCN)]
                    for dc in range(DC):
                        for hc in range(HCN):
                            nc.tensor.matmul(
                                ptiles[hc][:],
                                lhsT=w_sb[:, dc, ts(hc, 128)],
                                rhs=xb[:, dc, :],
                                start=(dc == 0),
                                stop=(dc == DC - 1),
                            )
                    for hc in range(HCN):
                        if has_bias:
                            nc.vector.tensor_scalar_add(
                                dst[:, hc, ts(tc_i, 512)], ptiles[hc][:], bq_sb[:, hc : hc + 1]
                            )
                        else:
                            nc.scalar.copy(dst[:, hc, ts(tc_i, 512)], ptiles[hc][:])

        # ---------------- phase B: v projection + attention ----------------
        if phases == "proj":
            # probe mode: just dump qhT/khT to out to have outputs written
            for qt in range(4):
                o_sb = opool.tile([128, 512], F32, tag="o", name="o_probe")
                nc.vector.tensor_copy(o_sb[:], qhT[:, qt, 0:512])
                nc.sync.dma_start(out_d[qt, :, 0:512], o_sb[:])
            return
        if phases == "attn":
            # probe mode: fill qhT/khT/vha with small garbage-free values
            nc.vector.memset(qhT[:], 0.01)
            nc.vector.memset(khT[:], 0.01)
            nc.vector.memset(vha[:], 0.01)
            nc.vector.memset(vha[:, :, :, D_V : D_V + 1], 1.0)
        with (
            tc.tile_pool(name="vstage", bufs=2) as vstage,
            tc.tile_pool(name="vstageb", bufs=2) as vstageb,
            tc.tile_pool(name="expool", bufs=20) as expool,
            tc.tile_pool(name="ps", bufs=2, space="PSUM") as ps,
            tc.tile_pool(name="cp", bufs=3, space="PSUM") as cp,
            tc.tile_pool(name="vp", bufs=1, space="PSUM") as vp,
        ):
            exp_tiles = {}

            def vh_proj():
                # vh (normal layout): psum (128 s-tile, 512 hd) = xT[dc, s].T @ W[dc]
                for sc in range(SCN):
                    xf = vstage.tile([128, DC, 128], F32, tag="vxf", name="vxf_t")
                    nc.sync.dma_start(xf[:], xv_d[:, :, ts(sc, 128)])
                    xb = vstageb.tile([128, DC, 128], BF16, tag="vxb", name="vxb_t")
                    nc.vector.tensor_copy(xb[:], xf[:])
                    pv = vp.tile([128, 512], F32, tag="vp", name="pv_t")
                    for dc in range(DC):
                        nc.tensor.matmul(
                            pv[:],
                            lhsT=xb[:, dc, :],
                            rhs=wv_sb[:, dc, :],
                            start=(dc == 0),
                            stop=(dc == DC - 1),
                        )
                    nc.vector.tensor_copy(
                        vha[:, sc, :, 0:D_V],
                        pv[:].rearrange("p (h d) -> p h d", d=D_V),
                    )

            def scores_exp_pair(p):
                # pair p = heads (2p, 2p+1) = hc p, partition halves 0/64.
                # One psum tile holds both heads for one 512-q chunk; the two
                # K=64 matmuls target row groups 0-63 / 64-127 and run
                # concurrently on the tiled PE array.
                for qc in range(QCN):
                    for sc in range(SCN):
                        s_ps = ps.tile([128, 1024], F32, tag="ps", name="s_ps")
                        for hl in range(2):
                            pb = hl * 64
                            nc.tensor.matmul(
                                s_ps[:, ts(hl, 512)],
                                lhsT=khT[pb : pb + 64, p, ts(sc, 128)],
                                rhs=qhT[pb : pb + 64, p, ts(qc, 512)],
                                start=True,
                                stop=True,
                            )
                        e = expool.tile([128, 1024], BF16, tag="exp", name="exp_t")
                        nc.scalar.activation(
                            e[:], s_ps[:], mybir.ActivationFunctionType.Exp, scale=0.125
                        )
                        exp_tiles[(p, sc, qc)] = e

            def ctx_pair(p):
                for qc in range(QCN):
                    for hl in range(2):
                        h = 2 * p + hl
                        pb = hl * 64
                        c_ps = cp.tile([128, 512], F32, tag="cp", name="c_ps")
                        for sc in range(SCN):
                            nc.tensor.matmul(
                                c_ps[0 : D_V + 1, :],
                                lhsT=vha[:, sc, h, :],
                                rhs=exp_tiles[(p, sc, qc)][:, ts(hl, 512)],
                                start=(sc == 0),
                                stop=(sc == SCN - 1),
                            )
                        rz = zpool.tile([1, 512], F32, tag="rz", name="rz_t")
                        nc.vector.reciprocal(rz[:], c_ps[D_V : D_V + 1, :])
                        # partition-broadcast 1/Z across 64 rows on GPSIMD (idle)
                        bc_sb = zpool.tile([64, 512], F32, tag="bcs", name="bc_sb")
                        nc.gpsimd.partition_broadcast(bc_sb[:], rz[:], channels=64)
                        nc.vector.tensor_mul(
                            ctxT[pb : pb + 64, p, ts(qc, 512)], c_ps[0:D_V, :], bc_sb[:]
                        )
                    for sc in range(SCN):
                        del exp_tiles[(p, sc, qc)]

            vh_proj()
            for p in range(NH // 2):
                scores_exp_pair(p)
                if p > 0:
                    ctx_pair(p - 1)
            ctx_pair(NH // 2 - 1)

            # ---------------- output projection ----------------
            for qt in range(SCN):
                potiles = [
                    cp.tile([128, 512], F32, tag="cp", name="po_t") for _ in range(2)
                ]
                for hc in range(HCN):
                    for d2 in range(2):
                        nc.tensor.matmul(
                            potiles[d2][:],
                            lhsT=ctxT[:, hc, ts(qt, 128)],
                            rhs=wo_sb[:, hc, ts(d2, 512)],
                            start=(hc == 0),
                            stop=(hc == HCN - 1),
                        )
                for d2 in range(2):
                    o_sb = opool.tile([128, 512], F32, tag="o", name="o_sb")
                    nc.vector.tensor_copy(o_sb[:], potiles[d2][:])
                    nc.sync.dma_start(out_d[qt, :, ts(d2, 512)], o_sb[:])


# ---------------------------------------------------------------------------
# host side
# ---------------------------------------------------------------------------

_NC_CACHE = {}


def _get_nc(reps: int = 1):
    if reps not in _NC_CACHE:
        _NC_CACHE[reps] = build_nc(reps)
    return _NC_CACHE[reps]


def _to_bf16(a):
    return np.ascontiguousarray(a).astype(ml_dtypes.bfloat16)


def make_in_maps(q, k, v, Wq, bq, Wk, bk, Wv, bv, Wo, bo):
    """Build the per-core input maps (host-side sharding + layout)."""
    in_maps = []
    for c in range(N_CORES):
        b = c // 2
        hg = c % 2
        hs = slice(hg * NH, hg * NH + NH)

        def xt(x):
            # (S, D) -> [p, dc, t] with D = dc*128 + p
            return np.ascontiguousarray(
                np.asarray(x, np.float32).T.reshape(DC, 128, T).transpose(1, 0, 2)
            )

        def wproj(W):
            # (8, 1024, 64) -> [p, dc, hd]  (hd = h*64+dv, D = dc*128+p)
            Wc = np.asarray(W[hs], np.float32).transpose(1, 0, 2).reshape(D_MODEL, HD)
            return _to_bf16(Wc.reshape(DC, 128, HD).transpose(1, 0, 2))

        wo_c = np.asarray(Wo[hg * HD : (hg + 1) * HD], np.float32)  # (512, 1024)
        bq_c = np.asarray(bq[hs], np.float32).reshape(HD)  # (512,)

        in_maps.append(
            {
                "xq_t": xt(q[b]),
                "xk_t": xt(k[b]),
                "xv_t": xt(v[b]),
                "wq": wproj(Wq),
                "wk": wproj(Wk),
                "wv": wproj(Wv),
                "wo": _to_bf16(wo_c.reshape(HCN, 128, D_MODEL).transpose(1, 0, 2)),
                "bq": np.ascontiguousarray(bq_c.reshape(HCN, 128).T),
            }
        )
    return in_maps


def combine_outputs(results, bv, Wo, bo):
    """results: list of 8 dicts with 'out' (16,128,1024). Returns (B,S,D)."""
    bo_eff = np.asarray(bo, np.float32) + np.asarray(bv, np.float32).reshape(-1) @ np.asarray(
        Wo, np.float32
    )
    out = np.empty((B, S, D_MODEL), np.float32)
    for b in range(B):
        p0 = results[2 * b]["out"].reshape(S, D_MODEL)
        p1 = results[2 * b + 1]["out"].reshape(S, D_MODEL)
        out[b] = p0 + p1 + bo_eff
    return out


def kernel(q, k, v, Wq, bq, Wk, bk, Wv, bv, Wo, bo):
    from concourse.bass_utils import run_bass_kernel_spmd

    nc = _get_nc(1)
    in_maps = make_in_maps(q, k, v, Wq, bq, Wk, bk, Wv, bv, Wo, bo)
    res = run_bass_kernel_spmd(nc, in_maps, core_ids=list(range(N_CORES)))
    return combine_outputs(res.results, bv, Wo, bo)


# revision 18
# speedup vs baseline: 1.6144x; 1.0089x over previous
"""Multi-head attention Trainium2 kernel.

Problem: B=4, S=2048, D_MODEL=1024, H=16 heads, d_k=d_v=64.

Sharding (8 cores, no collectives): core c handles batch b=c//2 and head
group g=c%2 (8 heads). Each core computes its 8 heads' attention and the
partial output projection ctx @ Wo[g's rows]; the host sums the two
head-group partials per batch and adds the (folded) biases.

Math notes:
 - bk drops out of softmax exactly (adds a per-(q,head) constant to every
   score in a row).
 - bv and bo fold into a single host-side row vector: bo_eff = bo + bv@Wo
   (softmax weights sum to 1).
 - softmax is computed without max subtraction: weights are ~N(0, 0.02^2),
   so scores have std ~0.4 and |score| < ~3; exp is safe in fp32.
 - matmuls run in bf16 with fp32 PSUM accumulation.

Device layout (per core):
 - scores^T[s, q] per head: s on partitions (16 tiles of 128), q on free.
   lhsT = khT (dk x s-chunk), rhs = qhT (dk x q) -> K=64 matmuls; head
   pairs sit at partition halves 0-63 / 64-127 so pairs run concurrently
   on the row-tiled PE array.
 - exp on ScalarE reads scores PSUM (128x2048) and writes bf16 tiles.
 - ctx^T via lhsT = [vh | ones] (128 s-chunk x 65): row 64 of the PSUM
   output is the softmax denominator Z[q], obtained for free.
 - 1/Z broadcast across 64 partitions on GPSIMD (idle otherwise); VectorE
   multiplies ctx rows and writes bf16 ctxT.
 - out[q, :] = sum_hc ctxT[:, hc, q-tile].T @ Wo chunk, fp32 out to HBM.
"""

import numpy as np
import ml_dtypes

import concourse.bass as bass
import concourse.bacc as bacc
import concourse.mybir as mybir
import concourse.tile as tile
from concourse.bass import ts

BF16 = mybir.dt.bfloat16
F32 = mybir.dt.float32

D_MODEL, D_K, D_V, N_HEADS = 1024, 64, 64, 16
B, S = 4, 2048
N_CORES = 8
NH = 8            # heads per core
HD = NH * D_V     # 512, stacked head dim per core
T = S             # tokens per core (one batch)
DC = 8            # D_MODEL / 128 chunks
TCN = 4           # token chunks of 512 for projections
SCN = 16          # s tiles of 128
QCN = 4           # q chunks of 512
HCN = 4           # hd chunks of 128 (2 heads each)
EXP_BUFS = 20
PS_BUFS = 2
CP_BUFS = 3


def build_nc(reps: int = 1, phases: str = "all"):
    """Build the per-core Bass module. reps>1 wraps the body in a HW loop
    (used only for timing). phases in {"all","proj","attn"} for perf probes."""
    nc = bacc.Bacc("TRN2", target_bir_lowering=False, debug=False)

    xq_d = nc.dram_tensor("xq_t", [128, DC, T], F32, kind="ExternalInput")
    xk_d = nc.dram_tensor("xk_t", [128, DC, T], F32, kind="ExternalInput")
    xv_d = nc.dram_tensor("xv_t", [128, DC, T], F32, kind="ExternalInput")
    wq_d = nc.dram_tensor("wq", [128, DC, HD], BF16, kind="ExternalInput")
    wk_d = nc.dram_tensor("wk", [128, DC, HD], BF16, kind="ExternalInput")
    wv_d = nc.dram_tensor("wv", [128, DC, HD], BF16, kind="ExternalInput")
    wo_d = nc.dram_tensor("wo", [128, HCN, D_MODEL], BF16, kind="ExternalInput")
    bq_d = nc.dram_tensor("bq", [128, HCN], F32, kind="ExternalInput")
    out_d = nc.dram_tensor("out", [SCN, 128, D_MODEL], F32, kind="ExternalOutput")

    with tile.TileContext(nc) as tc:
        def body():
            emit_body(nc, tc, xq_d, xk_d, xv_d, wq_d, wk_d, wv_d, wo_d, bq_d, out_d, phases)

        if reps == 1:
            body()
        else:
            with tc.For_i(0, reps, 1):
                body()
    nc.compile()
    return nc


def emit_body(nc, tc, xq_d, xk_d, xv_d, wq_d, wk_d, wv_d, wo_d, bq_d, out_d, phases="all"):
    import contextlib

    ctx = contextlib.ExitStack()
    with ctx:
        # ---------------- persistent SBUF pools ----------------
        wpool = ctx.enter_context(tc.tile_pool(name="wpool", bufs=1))
        qkpool = ctx.enter_context(tc.tile_pool(name="qkpool", bufs=1))
        vpool = ctx.enter_context(tc.tile_pool(name="vpool", bufs=1))
        cpool = ctx.enter_context(tc.tile_pool(name="cpool", bufs=1))
        opool = ctx.enter_context(tc.tile_pool(name="opool", bufs=4))
        zpool = ctx.enter_context(tc.tile_pool(name="zpool", bufs=4))

        wq_sb = wpool.tile([128, DC, HD], BF16, tag="wq")
        wk_sb = wpool.tile([128, DC, HD], BF16, tag="wk")
        wv_sb = wpool.tile([128, DC, HD], BF16, tag="wv")
        wo_sb = wpool.tile([128, HCN, D_MODEL], BF16, tag="wo")
        bq_sb = wpool.tile([128, HCN], F32, tag="bq")
        ones64 = wpool.tile([1, 64], BF16, tag="ones64")

        nc.sync.dma_start(wq_sb[:], wq_d[:])
        nc.sync.dma_start(wk_sb[:], wk_d[:])
        nc.sync.dma_start(wv_sb[:], wv_d[:])
        nc.sync.dma_start(wo_sb[:], wo_d[:])
        nc.sync.dma_start(bq_sb[:], bq_d[:])
        nc.vector.memset(ones64[:], 1.0)

        qhT = qkpool.tile([128, HCN, T], BF16, tag="qhT")  # [hd%128, hd//128, t]
        khT = qkpool.tile([128, HCN, T], BF16, tag="khT")
        vha = vpool.tile([128, SCN, NH, D_V + 1], BF16, tag="vha")  # [s%128, s//128, h, dv|1]
        ctxT = cpool.tile([128, HCN, T], BF16, tag="ctxT")

        nc.vector.memset(vha[:, :, :, D_V : D_V + 1], 1.0)

        # ---------------- phase A: q/k projections ----------------
        if phases in ("all", "proj"):
         with (
            tc.tile_pool(name="xf32", bufs=2) as xf32p,
            tc.tile_pool(name="xbf", bufs=2) as xbfp,
            tc.tile_pool(name="pp", bufs=6, space="PSUM") as pp,
        ):
            # qhT / khT: psum (128 hd-chunk, 512 t) = W[dc,hc].T @ xT[dc, tc]
            for x_d, w_sb, dst, has_bias in (
                (xq_d, wq_sb, qhT, True),
                (xk_d, wk_sb, khT, False),
            ):
                for tc_i in range(TCN):
                    xf = xf32p.tile([128, DC, 512], F32, tag="xf", name="xf_t")
                    nc.sync.dma_start(xf[:], x_d[:, :, ts(tc_i, 512)])
                    xb = xbfp.tile([128, DC, 512], BF16, tag="xb", name="xb_t")
                    nc.vector.tensor_copy(xb[:], xf[:])
                    ptiles = [pp.tile([128, 512], F32, tag="pp", name="pp_t") for _ in range(HCN)]
                    for dc in range(DC):
                        for hc in range(HCN):
                            nc.tensor.matmul(
                                ptiles[hc][:],
                                lhsT=w_sb[:, dc, ts(hc, 128)],
                                rhs=xb[:, dc, :],
                                start=(dc == 0),
                                stop=(dc == DC - 1),
                            )
                    for hc in range(HCN):
                        if has_bias:
                            nc.vector.tensor_scalar_add(
                                dst[:, hc, ts(tc_i, 512)], ptiles[hc][:], bq_sb[:, hc : hc + 1]
                            )
                        else:
                            nc.scalar.copy(dst[:, hc, ts(tc_i, 512)], ptiles[hc][:])

        # ---------------- phase B: v projection + attention ----------------
        if phases == "proj":
            # probe mode: just dump qhT/khT to out to have outputs written
            for qt in range(4):
                o_sb = opool.tile([128, 512], F32, tag="o", name="o_probe")
                nc.vector.tensor_copy(o_sb[:], qhT[:, qt, 0:512])
                nc.sync.dma_start(out_d[qt, :, 0:512], o_sb[:])
            return
        if phases in ("attn", "sx"):
            # probe mode: fill qhT/khT/vha with small garbage-free values
            nc.vector.memset(qhT[:], 0.01)
            nc.vector.memset(khT[:], 0.01)
            nc.vector.memset(vha[:], 0.01)
            nc.vector.memset(vha[:, :, :, D_V : D_V + 1], 1.0)
        with (
            tc.tile_pool(name="vstage", bufs=2) as vstage,
            tc.tile_pool(name="vstageb", bufs=2) as vstageb,
            tc.tile_pool(name="expool", bufs=EXP_BUFS) as expool,
            tc.tile_pool(name="ps", bufs=PS_BUFS, space="PSUM") as ps,
            tc.tile_pool(name="cp", bufs=CP_BUFS, space="PSUM") as cp,
            tc.tile_pool(name="vp", bufs=1, space="PSUM") as vp,
        ):
            exp_tiles = {}

            def vh_proj():
                # vh (normal layout): psum (128 s-tile, 512 hd) = xT[dc, s].T @ W[dc]
                for sc in range(SCN):
                    xf = vstage.tile([128, DC, 128], F32, tag="vxf", name="vxf_t")
                    nc.sync.dma_start(xf[:], xv_d[:, :, ts(sc, 128)])
                    xb = vstageb.tile([128, DC, 128], BF16, tag="vxb", name="vxb_t")
                    nc.vector.tensor_copy(xb[:], xf[:])
                    pv = vp.tile([128, 512], F32, tag="vp", name="pv_t")
                    for dc in range(DC):
                        nc.tensor.matmul(
                            pv[:],
                            lhsT=xb[:, dc, :],
                            rhs=wv_sb[:, dc, :],
                            start=(dc == 0),
                            stop=(dc == DC - 1),
                        )
                    nc.vector.tensor_copy(
                        vha[:, sc, :, 0:D_V],
                        pv[:].rearrange("p (h d) -> p h d", d=D_V),
                    )

            def scores_exp_pair(p):
                # pair p = heads (2p, 2p+1) = hc p, partition halves 0/64.
                # One psum tile holds both heads for one 512-q chunk; the two
                # K=64 matmuls target row groups 0-63 / 64-127 and run
                # concurrently on the tiled PE array.
                for qc in range(QCN):
                    for sc in range(SCN):
                        s_ps = ps.tile([128, 1024], F32, tag="ps", name="s_ps")
                        for hl in range(2):
                            pb = hl * 64
                            nc.tensor.matmul(
                                s_ps[:, ts(hl, 512)],
                                lhsT=khT[pb : pb + 64, p, ts(sc, 128)],
                                rhs=qhT[pb : pb + 64, p, ts(qc, 512)],
                                start=True,
                                stop=True,
                            )
                        e = expool.tile([128, 1024], BF16, tag="exp", name="exp_t")
                        nc.scalar.activation(
                            e[:], s_ps[:], mybir.ActivationFunctionType.Exp, scale=0.125
                        )
                        exp_tiles[(p, sc, qc)] = e

            def ctx_pair(p):
                for qc in range(QCN):
                    for hl in range(2):
                        h = 2 * p + hl
                        pb = hl * 64
                        c_ps = cp.tile([128, 512], F32, tag="cp", name="c_ps")
                        for sc in range(SCN):
                            nc.tensor.matmul(
                                c_ps[0 : D_V + 1, :],
                                lhsT=vha[:, sc, h, :],
                                rhs=exp_tiles[(p, sc, qc)][:, ts(hl, 512)],
                                start=(sc == 0),
                                stop=(sc == SCN - 1),
                            )
                        rz = zpool.tile([1, 512], F32, tag="rz", name="rz_t")
                        nc.vector.reciprocal(rz[:], c_ps[D_V : D_V + 1, :])
                        # partition-broadcast 1/Z across 64 rows on GPSIMD (idle)
                        bc_sb = zpool.tile([64, 512], F32, tag="bcs", name="bc_sb")
                        nc.gpsimd.partition_broadcast(bc_sb[:], rz[:], channels=64)
                        nc.vector.tensor_mul(
                            ctxT[pb : pb + 64, p, ts(qc, 512)], c_ps[0:D_V, :], bc_sb[:]
                        )
                    for sc in range(SCN):
                        del exp_tiles[(p, sc, qc)]

            if phases == "sx":
                # probe: scores+exp only
                for p in range(NH // 2):
                    scores_exp_pair(p)
                    for key in list(exp_tiles):
                        del exp_tiles[key]
                for qt in range(4):
                    o_sb = opool.tile([128, 512], F32, tag="o", name="o_probe2")
                    nc.vector.tensor_copy(o_sb[:], qhT[:, qt, 0:512])
                    nc.sync.dma_start(out_d[qt, :, 0:512], o_sb[:])
                return
            vh_proj()
            for p in range(NH // 2):
                scores_exp_pair(p)
                if p > 0:
                    ctx_pair(p - 1)
            ctx_pair(NH // 2 - 1)

            # ---------------- output projection ----------------
            for qt in range(SCN):
                potiles = [
                    cp.tile([128, 512], F32, tag="cp", name="po_t") for _ in range(2)
                ]
                for hc in range(HCN):
                    for d2 in range(2):
                        nc.tensor.matmul(
                            potiles[d2][:],
                            lhsT=ctxT[:, hc, ts(qt, 128)],
                            rhs=wo_sb[:, hc, ts(d2, 512)],
                            start=(hc == 0),
                            stop=(hc == HCN - 1),
                        )
                for d2 in range(2):
                    o_sb = opool.tile([128, 512], F32, tag="o", name="o_sb")
                    nc.scalar.copy(o_sb[:], potiles[d2][:])
                    nc.sync.dma_start(out_d[qt, :, ts(d2, 512)], o_sb[:])


# ---------------------------------------------------------------------------
# host side
# ---------------------------------------------------------------------------

_NC_CACHE = {}


def _get_nc(reps: int = 1):
    if reps not in _NC_CACHE:
        _NC_CACHE[reps] = build_nc(reps)
    return _NC_CACHE[reps]


def _to_bf16(a):
    return np.ascontiguousarray(a).astype(ml_dtypes.bfloat16)


def make_in_maps(q, k, v, Wq, bq, Wk, bk, Wv, bv, Wo, bo):
    """Build the per-core input maps (host-side sharding + layout)."""
    in_maps = []
    for c in range(N_CORES):
        b = c // 2
        hg = c % 2
        hs = slice(hg * NH, hg * NH + NH)

        def xt(x):
            # (S, D) -> [p, dc, t] with D = dc*128 + p
            return np.ascontiguousarray(
                np.asarray(x, np.float32).T.reshape(DC, 128, T).transpose(1, 0, 2)
            )

        def wproj(W):
            # (8, 1024, 64) -> [p, dc, hd]  (hd = h*64+dv, D = dc*128+p)
            Wc = np.asarray(W[hs], np.float32).transpose(1, 0, 2).reshape(D_MODEL, HD)
            return _to_bf16(Wc.reshape(DC, 128, HD).transpose(1, 0, 2))

        wo_c = np.asarray(Wo[hg * HD : (hg + 1) * HD], np.float32)  # (512, 1024)
        bq_c = np.asarray(bq[hs], np.float32).reshape(HD)  # (512,)

        in_maps.append(
            {
                "xq_t": xt(q[b]),
                "xk_t": xt(k[b]),
                "xv_t": xt(v[b]),
                "wq": wproj(Wq),
                "wk": wproj(Wk),
                "wv": wproj(Wv),
                "wo": _to_bf16(wo_c.reshape(HCN, 128, D_MODEL).transpose(1, 0, 2)),
                "bq": np.ascontiguousarray(bq_c.reshape(HCN, 128).T),
            }
        )
    return in_maps


def combine_outputs(results, bv, Wo, bo):
    """results: list of 8 dicts with 'out' (16,128,1024). Returns (B,S,D)."""
    bo_eff = np.asarray(bo, np.float32) + np.asarray(bv, np.float32).reshape(-1) @ np.asarray(
        Wo, np.float32
    )
    out = np.empty((B, S, D_MODEL), np.float32)
    for b in range(B):
        p0 = results[2 * b]["out"].reshape(S, D_MODEL)
        p1 = results[2 * b + 1]["out"].reshape(S, D_MODEL)
        out[b] = p0 + p1 + bo_eff
    return out


def kernel(q, k, v, Wq, bq, Wk, bk, Wv, bv, Wo, bo):
    from concourse.bass_utils import run_bass_kernel_spmd

    nc = _get_nc(1)
    in_maps = make_in_maps(q, k, v, Wq, bq, Wk, bk, Wv, bv, Wo, bo)
    res = run_bass_kernel_spmd(nc, in_maps, core_ids=list(range(N_CORES)))
    return combine_outputs(res.results, bv, Wo, bo)
